# revision 31
# baseline (speedup 1.0000x reference)
"""MobileAttention3D Trainium2 kernel (8-core SPMD), v3.

Sharding: core c -> (b = c//4, hg = c%4) owns batch b and H rows
[8*hg, 8*hg+8).  All conv GEMMs + attention for that slice run locally;
the only cross-core communication is a 32KB AllReduce of partial
attention logits within each batch group {0..3}, {4..7}.

v3 changes vs v2 (trace-driven; ~158us -> ~142-150us):
  * CC warmup: dummy 64B 2-rank AllReduce triggered at t~0 absorbs the
    collective stream's first-op cold cost (~12us extra duration) under
    the conv phase; the whole AR chain (arin/l3 DMAs + triggers) lives
    on the otherwise-idle gpsimd queue so the l3 readback fires the
    instant an AR completes.
  * Q2 relayout to (eta, kd, dq, n) and ksb to (eta, kd, dk): both
    logits operands stream contiguously (strided rhs ran the PE at
    ~1/4 rate, 234ns vs 66ns per 128-col stream).
  * q-conv drains merged to [P,1024] two-bank copies; proj psum merged
    to [P,1024] (both ct halves), single drain.
  * software-pipelined tail: the AV matmul groups of unit k+1 issue
    interleaved with the proj quads of unit k (units = (mu, eta)), so
    the PE streams proj while the AV drains flow and vice versa.
  * softmax without max-subtraction (|logit*scale| < ~25, fp32-exp
    safe) shortens the AR -> first-AV critical chain.
  * DMA trigger hygiene: scalar/vector stay drain-only where possible;
    out stores on sync; final flush drains both engines in parallel
    and splits the last store 4 ways.
Remaining span is PE-throughput-bound (~110us of matmul streaming at
the ~1.95GHz throttled clock) plus the NEFF CC-init barrier
(10-40us run-to-run) + ~30us serial AR stream latency, largely
overlapped.  Output token order is (w', dq, n); host unshard adapts.
"""

import numpy as np
import ml_dtypes

NH, KD, VD, C = 8, 64, 64, 256
B, D, H, W = 2, 32, 32, 32
HS = H // 4            # h rows per core
T = D * HS * W         # 8192 tokens per core
P = 128
NCORES = 8
SCALE = float(VD) ** -0.5

_CACHE = {}


def _build(has_qb, has_kvb, has_pb, sim_mode=False):
    import concourse.bacc as bacc
    import concourse.mybir as mybir
    from concourse import tile

    dt = mybir.dt
    f32, bf16 = dt.float32, dt.bfloat16
    AX = mybir.AxisListType
    AF = mybir.ActivationFunctionType

    nc = bacc.Bacc("TRN2", target_bir_lowering=False, debug=False,
                   enable_asserts=False,
                   num_devices=1 if sim_mode else NCORES)

    x_in = nc.dram_tensor("x", [C, T], bf16, kind="ExternalInput")
    wq_in = nc.dram_tensor("wq", [C, NH * KD], bf16, kind="ExternalInput")
    wkv_in = nc.dram_tensor("wkv", [C, KD + VD], bf16, kind="ExternalInput")
    wp_in = nc.dram_tensor("wp", [NH * VD, C], bf16, kind="ExternalInput")
    idt_in = nc.dram_tensor("idt", [P, P], bf16, kind="ExternalInput")
    idtf_in = nc.dram_tensor("idtf", [P, 32], f32, kind="ExternalInput")
    qb_in = kvb_in = pb_in = None
    if has_qb:
        qb_in = nc.dram_tensor("qb", [P, NH * KD], bf16, kind="ExternalInput")
    if has_kvb:
        kvb_in = nc.dram_tensor("kvb", [P, KD + VD], bf16, kind="ExternalInput")
    if has_pb:
        # proj bias pre-multiplied by layer_scale, per C channel
        pb_in = nc.dram_tensor("pb", [C, 1], f32, kind="ExternalInput")
    out_t = nc.dram_tensor("out", [C, T], f32, kind="ExternalOutput")

    with tile.TileContext(nc) as tc:
        with tc.tile_pool(name="wpool", bufs=1) as wpool, \
             tc.tile_pool(name="big", bufs=1) as bigpool, \
             tc.tile_pool(name="q2p", bufs=1) as q2pool, \
             tc.tile_pool(name="kvp", bufs=1) as kvpool, \
             tc.tile_pool(name="small", bufs=1) as spool, \
             tc.tile_pool(name="stage", bufs=4) as stpool, \
             tc.tile_pool(name="psA", bufs=4, space="PSUM") as psum, \
             tc.tile_pool(name="psB", bufs=2, space="PSUM") as psum2, \
             tc.tile_pool(name="dram", bufs=1, space="DRAM") as dram:

            # engine rotation for PSUM evacuation copies.
            # (GPSIMD cannot access PSUM, so only vector+scalar rotate.)
            rot_engines = [nc.vector, nc.scalar]
            rot_state = [0]

            def rot_copy(dst, src):
                eng = rot_engines[rot_state[0] % 2]
                rot_state[0] += 1
                if eng is nc.scalar:
                    eng.copy(dst, src)
                else:
                    eng.tensor_copy(dst, src)

            def rot_tt(dst, a, b_, op):
                nc.vector.tensor_tensor(dst, a, b_, op=op)

            # ---- CC warmup: tiny AllReduce with no data deps, triggered
            # at t~0.  The collective stream's first op pays ~12us extra
            # duration (cold ncfw/descriptor staging) plus the NEFF-init
            # barrier wait; burn both on a dummy while the convs run so
            # the real logit AR runs at steady-state latency.  Must
            # outrank the hp logit-AR trigger on the gpsimd queue: ncfw
            # serves collectives in staging order. ----
            if not sim_mode:
                prio = tc.cur_priority
                tc.cur_priority = -1000
                warm_sb = spool.tile([1, 16], f32, name="warm_sb")
                warm_in = dram.tile([1, 16], f32, name="warm_in")
                warm_out = dram.tile([1, 16], f32, name="warm_out")
                nc.gpsimd.memset(warm_sb[:], 0.0)
                nc.gpsimd.dma_start(warm_in[:], warm_sb[:])
                # 2-rank groups: cheapest op that still boots the CC
                # stream (the cold surcharge is stream-level, not per-op)
                nc.gpsimd.collective_compute(
                    "AllReduce", mybir.AluOpType.add,
                    replica_groups=[[0, 1], [2, 3], [4, 5], [6, 7]],
                    ins=[warm_in.opt()], outs=[warm_out.opt()])
                tc.cur_priority = prio

            # ---- load weights / constants ----
            wq = wpool.tile([P, 2, NH * KD], bf16)
            wkv = wpool.tile([P, 2, KD + VD], bf16)
            wp = wpool.tile([P, 4, C], bf16)
            idt = wpool.tile([P, P], bf16)
            idtf = wpool.tile([P, 32], f32)
            # wkv is needed first (kv m0); wq only after m0's kv matmuls,
            # so it queues behind the first x chunk (issued below)
            for ci in range(2):
                nc.sync.dma_start(wkv[:, ci, :], wkv_in[ci * P:(ci + 1) * P, :])
            qb = kvb = pb = None
            if has_qb:
                qb = wpool.tile([P, NH * KD], bf16)
                nc.sync.dma_start(qb[:], qb_in[:])
            if has_kvb:
                kvb = wpool.tile([P, KD + VD], bf16)
                nc.sync.dma_start(kvb[:], kvb_in[:])
            if has_pb:
                pb = wpool.tile([P, 2, 1], f32)
                for ci in range(2):
                    nc.sync.dma_start(pb[:, ci, :], pb_in[ci * P:(ci + 1) * P, :])

            # big slot shared sequentially: x (32KB/p) then oo (64KB/p)
            # coarse x chunks: DMA ring throughput is descriptor-rate
            # bound (~130ns/descriptor), so 4KB-per-partition runs beat
            # 1KB ones 4x; 4 chunks still pace the 16-step m-loop fine.
            x_sb = bigpool.tile([P, 2, T], bf16, tag="big")
            XCH = 16
            for g in range(XCH):
                lo, hi = g * (T // XCH), (g + 1) * (T // XCH)
                for ci in range(2):
                    eng = nc.sync if ci == 0 else nc.scalar
                    eng.dma_start(x_sb[:, ci, lo:hi],
                                  x_in[ci * P:(ci + 1) * P, lo:hi])
                if g == 0:
                    for ci in range(2):
                        nc.sync.dma_start(wq[:, ci, :],
                                          wq_in[ci * P:(ci + 1) * P, :])
            nc.sync.dma_start(idt[:], idt_in[:])
            nc.sync.dma_start(idtf[:], idtf_in[:])
            for jq in range(4):
                nc.sync.dma_start(wp[:, jq, :], wp_in[jq * P:(jq + 1) * P, :])

            # Q2 [p=hw128, (eta, kd, dq, n)] -- the logits rhs for a
            # given (eta, kd, mu) is then a contiguous 128-col stream
            # (strided streams ran the PE at ~1/4 rate)
            Q2 = q2pool.tile([P, 32 * 1024], bf16)
            ksb = kvpool.tile([P, 64 * KD], bf16)      # [p=hw128, (eta, kd, dk)]
            vsb = kvpool.tile([P, 64 * VD], bf16)      # [p=hw128, (dk, eta, vd)]
            # vatt4: strip r=[32r..32r+32) holds [dk, (eta, q, b, vd)] for
            # hw128 in [32r, 32r+32);  q = hw128%32 // 2, b = hw128%2
            vatt4 = kvpool.tile([P, 2 * 16 * 2 * VD], bf16)
            attn = spool.tile([P, 2, 32], bf16)
            attnT4 = spool.tile([P, 2, P], bf16)       # attn^T replicated 4 strips
            l2s = spool.tile([P, 256], f32)            # logits strips (dk, nq')
            lsum0 = spool.tile([P, 2, 32], f32)
            lsum1 = spool.tile([P, 2, 32], f32)
            l2 = spool.tile([P, 64], f32)
            l3 = spool.tile([P, 64], f32)
            ex = spool.tile([P, 2, 32], f32)
            red = spool.tile([P, 8], f32)

            arin = [dram.tile([P, 32], f32, name=f"arin{mu}")
                    for mu in range(2)]
            arout = [dram.tile([P, 32], f32, name=f"arout{mu}")
                     for mu in range(2)]

            # ---- kv + q convs (tokens on partitions), tracking x DMA
            # arrival.  q conv for all 32 dq rides the m-loop so both
            # logit halves are ready as early as possible. ----
            Q2f = Q2.rearrange("p (e k dq n) -> p e k dq n",
                               e=2, k=KD, dq=32, n=NH)

            def q_conv(dq):
                # one 2-bank psum tile for both eta halves; drain is a
                # single strided copy into the (eta, kd, dq, n) layout.
                psq = psum2.tile([P, 1024], f32, tag="pp", name=f"psq{dq}")
                for eta in range(2):
                    j = dq * 2 + eta
                    for ci in range(2):
                        nc.tensor.matmul(psq[:, eta * 512:(eta + 1) * 512],
                                         x_sb[:, ci, j * P:(j + 1) * P],
                                         wq[:, ci, :],
                                         start=(ci == 0), stop=(ci == 1))
                dst = Q2f[:, :, :, dq, :]
                src = psq.rearrange("p (e k n) -> p e k n", e=2, k=KD)
                if has_qb:
                    rot_tt(dst, src,
                           qb.rearrange("p (k n) -> p 1 k n", k=KD)[:, [0, 0]],
                           mybir.AluOpType.add)
                else:
                    rot_copy(dst, src)

            for m in range(16):
                ps = psum.tile([P, 512], f32, tag="ps", name=f"pskv{m}")
                for jj in range(4):
                    j = 4 * m + jj
                    for ci in range(2):
                        nc.tensor.matmul(
                            ps[:, jj * P:(jj + 1) * P],
                            x_sb[:, ci, j * P:(j + 1) * P],
                            wkv[:, ci, :],
                            start=(ci == 0), stop=(ci == 1))
                psv = ps.rearrange("p (t c) -> p t c", c=P)
                # k into (eta, kd, dk) [contiguous logits lhsT]; the psum
                # t-chunks are (dd, eta)-ordered, so view both sides as
                # [p, e, kd, dd] 4D APs.
                ks = ksb.rearrange("p (e k d) -> p e k d",
                                   e=2, k=KD)[:, :, :, 2 * m:2 * m + 2]
                kss = ps.rearrange("p (dd e c) -> p e c dd",
                                   dd=2, e=2)[:, :, 0:KD, :]
                vs = vsb[:, m * 256:(m + 1) * 256].rearrange("p (t c) -> p t c", c=VD)
                if has_kvb:
                    kvbv = kvb.rearrange("p c -> p 1 c")
                    rot_tt(ks, kss,
                           kvb.rearrange("p c -> p 1 c 1")[:, [0, 0], 0:KD],
                           mybir.AluOpType.add)
                    rot_tt(vs, psv[:, :, KD:KD + VD],
                           kvbv[:, [0, 0, 0, 0], KD:KD + VD], mybir.AluOpType.add)
                else:
                    rot_copy(ks, kss)
                    rot_copy(vs, psv[:, :, KD:KD + VD])
                if m < 8:
                    # mu0's dq half rides the kv loop so logits-mu0/AR0
                    # can trigger right after the last kv chunk
                    q_conv(2 * m)
                    q_conv(2 * m + 1)

            # ---- v "transpose" into vatt4 via DMA (DRAM bounce so every
            # SBUF AP is partition-first); overlaps the q conv ----
            vtd = [dram.tile([2, 32, 2048], bf16, name=f"vtd{r}")
                   for r in range(4)]
            for r in range(4):
                src1 = vsb[32 * r:32 * (r + 1), :].rearrange(
                    "qb (k e v) -> e qb k v", k=32, e=2, v=VD)
                dst1 = vtd[r].rearrange("e k (qb v) -> e qb k v",
                                        qb=32, v=VD)
                for eta in range(2):
                    nc.sync.dma_start(dst1[eta], src1[eta])
            for r in range(4):
                for eta in range(2):
                    nc.sync.dma_start(
                        vatt4[32 * r:32 * (r + 1),
                              eta * 2048:(eta + 1) * 2048],
                        vtd[r][eta])

            # ---- per-mu: q conv half -> logits half -> AllReduce ----
            # nq' = dq*8+n, so mu = dq-half: logits/AR for mu0 launch after
            # only half the q conv; the serial CC stream then services AR0
            # while mu1's q conv + logits run.  high_priority: the AR
            # trigger chain must beat the other mu's q conv to the PE.
            Q2v = Q2f
            ksv = ksb.rearrange("p (e k d) -> p e k d", e=2, k=KD)
            for mu in range(2):
                if mu == 1:
                    for dq in range(16, 32):
                        q_conv(dq)
                hp = tc.high_priority()
                hp.__enter__()
                psL2 = psum.tile([P, P], f32, tag="ps", name=f"psL2_{mu}")
                for step in range(32):
                    for c_ in range(4):
                        idx = c_ * 32 + step
                        eta, kd = idx // KD, idx % KD
                        nc.tensor.matmul(
                            psL2[32 * c_:32 * (c_ + 1), :],
                            ksv[:, eta, kd, :],
                            Q2v[:, eta, kd, mu * 16:(mu + 1) * 16, :],
                            start=(step == 0), stop=(step == 31),
                            tile_position=(0, 32 * c_), skip_group_check=True)
                nc.vector.tensor_copy(l2s[:, mu * P:(mu + 1) * P], psL2[:])

                # strip sums + transpose to [nq', dk] via 4 fp32 row-tiles
                lt = [psum.tile([P, 32], f32, tag="ps", name=f"lt{mu}_{c_}")
                      for c_ in range(4)]
                for c_ in range(4):
                    nc.tensor.matmul(
                        lt[c_][:],
                        l2s[32 * c_:32 * (c_ + 1), mu * P:(mu + 1) * P],
                        idtf[32 * c_:32 * (c_ + 1), :],
                        start=True, stop=True,
                        tile_position=(32 * c_, 0))
                # <=1 PSUM operand per DVE op: stage lt0/lt2 through SBUF
                nc.vector.tensor_copy(lsum0[:, mu, :], lt[0][:])
                nc.scalar.copy(lsum1[:, mu, :], lt[2][:])
                nc.vector.tensor_tensor(lsum0[:, mu, :], lsum0[:, mu, :],
                                        lt[1][:], op=mybir.AluOpType.add)
                nc.vector.tensor_tensor(lsum1[:, mu, :], lsum1[:, mu, :],
                                        lt[3][:], op=mybir.AluOpType.add)
                nc.vector.tensor_tensor(l2[:, mu * 32:(mu + 1) * 32],
                                        lsum0[:, mu, :], lsum1[:, mu, :],
                                        op=mybir.AluOpType.add)
                # whole AR chain on the (otherwise idle) gpsimd queue so
                # the l3 readback fires the instant the AR completes,
                # instead of queueing behind tail out-store triggers.
                nc.gpsimd.dma_start(arin[mu][:], l2[:, mu * 32:(mu + 1) * 32])
                if sim_mode:
                    nc.gpsimd.dma_start(arout[mu][:], arin[mu][:])
                else:
                    nc.gpsimd.collective_compute(
                        "AllReduce", mybir.AluOpType.add,
                        replica_groups=[[0, 1, 2, 3], [4, 5, 6, 7]],
                        ins=[arin[mu].opt()], outs=[arout[mu].opt()])
                nc.gpsimd.dma_start(l3[:, mu * 32:(mu + 1) * 32], arout[mu][:])
                hp.__exit__(None, None, None)

            oo = bigpool.tile([P, 4, T], bf16, tag="big", name="oo")
            # oo free per jq plane: f' = w'*256 + nq',  nq' = dq*8 + n
            oov = oo.rearrange("p jq (wh wl n) -> p jq wl wh n", wh=8, wl=4)

            def softmax(mu):
                # ---- softmax over dk (free axis), then attn^T replicated
                # to 4 partition strips via a col-tiled matmul quad.
                # No max-subtraction: |logit*scale| < ~25 here, exp is
                # safe in fp32, and dropping it shortens the post-AR
                # critical chain by two engine hops. ----
                sl = l3[:, mu * 32:(mu + 1) * 32]
                sm = red[:, mu * 4 + 2: mu * 4 + 3]
                rs = red[:, mu * 4 + 3: mu * 4 + 4]
                nc.scalar.activation(ex[:, mu, :], sl, AF.Exp,
                                     scale=SCALE, accum_out=sm)
                nc.vector.reciprocal(rs, sm)
                nc.vector.tensor_scalar_mul(attn[:, mu, :], ex[:, mu, :], rs)
                psT = psum.tile([P, P], f32, tag="ps", name=f"psat{mu}")
                for r in range(4):
                    nc.tensor.matmul(psT[32 * r:32 * (r + 1), :],
                                     attn[:, mu, :], idt[:, 0:P],
                                     start=True, stop=True,
                                     tile_position=(0, 32 * r))
                nc.vector.tensor_copy(attnT4[:, mu, :], psT[:])

            def av_group(mu, eta, qh):
                # tile r holds strip r's outputs for all 4 jq (one PSUM
                # bank per row-tile; concurrent row-tiles never share one)
                pr = [psum.tile([P, 512], f32, tag="ps",
                                name=f"psav{mu}_{eta}_{qh}_{r}")
                      for r in range(4)]
                for jq in range(4):
                    q_ = qh * 4 + jq
                    for r in range(4):
                        nc.tensor.matmul(
                            pr[r][:, jq * P:(jq + 1) * P],
                            vatt4[32 * r:32 * (r + 1),
                                  eta * 2048 + q_ * P:eta * 2048 + (q_ + 1) * P],
                            attnT4[32 * r:32 * (r + 1), mu, :],
                            start=True, stop=True,
                            tile_position=(32 * r, 0))
                for r in range(4):
                    # [p, (jq, nq)] -> oo planes jq at w' = eta*16+4r+qh
                    rot_copy(
                        oov[:, :, qh, eta * 4 + r, mu * P:(mu + 1) * P],
                        pr[r].rearrange("p (jq n) -> p jq n", jq=4))

            # out token order t = mu*4096 + w'*128 + dqloc*8 + n
            outv = out_t.rearrange("(ct p) t -> p ct t", p=P)
            oow = [oo[:, jq, :].rearrange("p (w q) -> p w q", w=32)
                   for jq in range(4)]

            def proj_quad(mu, tq, last=False):
                # output tokens w' in [4tq, 4tq+4), nq'-half mu; both ct
                # halves in one 2-bank psum tile, single [P,1024] drain.
                stg = stpool.tile([P, 1024], f32, tag="stg",
                                  name=f"stg{mu}_{tq}")
                ps = psum2.tile([P, 1024], f32, tag="pp",
                                name=f"psp{mu}_{tq}")
                for ct in range(2):
                    for jq in range(4):
                        nc.tensor.matmul(
                            ps[:, ct * 512:(ct + 1) * 512],
                            wp[:, jq, ct * P:(ct + 1) * P],
                            oow[jq][:, 4 * tq:4 * (tq + 1),
                                    mu * P:(mu + 1) * P],
                            start=(jq == 0), stop=(jq == 3))
                if has_pb:
                    eng = rot_engines[rot_state[0] % 2]
                    rot_state[0] += 1
                    eng.tensor_scalar_add(
                        stg.rearrange("p (ct c) -> p ct c", ct=2),
                        ps.rearrange("p (ct c) -> p ct c", ct=2),
                        pb[:])
                elif last:
                    # flush path: both drain engines in parallel
                    nc.vector.tensor_copy(stg[:, 0:512], ps[:, 0:512])
                    nc.scalar.copy(stg[:, 512:1024], ps[:, 512:1024])
                else:
                    rot_copy(stg, ps[:])
                base = mu * 4096 + tq * 512
                stv = stg.rearrange("p (ct c) -> p ct c", ct=2)
                if last:
                    # final flush on the critical path: 4-way split so
                    # the last 512KB drains through parallel triggers
                    for ct in range(2):
                        for h in range(2):
                            eng = nc.sync if h == 0 else nc.scalar
                            eng.dma_start(
                                outv[:, ct, base + h * 256:base + (h + 1) * 256],
                                stv[:, ct, h * 256:(h + 1) * 256])
                else:
                    # sync engine only: scalar/vector are drain engines
                    nc.sync.dma_start(outv[:, :, base:base + 512], stv[:])

            # ---- software-pipelined tail over units (mu, eta):
            # av of unit k+1 interleaves with proj of unit k so the PE
            # streams proj while av drains flow. ----
            units = [(0, 0), (0, 1), (1, 0), (1, 1)]
            softmax(0)
            for qh in range(4):
                av_group(0, 0, qh)
            for i, (mu, eta) in enumerate(units):
                nxt = units[i + 1] if i + 1 < 4 else None
                for j in range(4):
                    if nxt is not None:
                        av_group(nxt[0], nxt[1], j)
                    proj_quad(mu, eta * 4 + j,
                              last=(i == 3 and j == 3))
                if i == 0:
                    # issue mu1's softmax one full unit before av(1,0)
                    # needs attnT4: its exp/recip/transpose ops then ride
                    # ahead of a unit of drain backlog instead of
                    # executing serially inside a PE stall.
                    softmax(1)

    nc.finalize()
    return nc


def _get_nc(has_qb, has_kvb, has_pb, sim_mode=False):
    key = (has_qb, has_kvb, has_pb, sim_mode)
    if key not in _CACHE:
        _CACHE[key] = _build(*key)
    return _CACHE[key]


def _host_inputs(q_w, q_b, kv_w, kv_b, proj_w, proj_b, layer_scale,
                 has_qb, has_kvb, has_pb):
    bf = ml_dtypes.bfloat16
    ls_c = layer_scale.reshape(C)                          # [C] f32
    # wq columns reordered to (kd, n) so the q-conv drain is contiguous
    wq = np.ascontiguousarray(
        q_w.reshape(NH, KD, C).transpose(2, 1, 0).reshape(C, NH * KD)
    ).astype(bf)
    wkv = np.ascontiguousarray(kv_w.T).astype(bf)          # [C, 128]
    wp = np.ascontiguousarray((proj_w * ls_c[:, None]).T).astype(bf)
    idt = np.eye(P, dtype=bf)
    idtf = np.tile(np.eye(32, dtype=np.float32), (4, 1))   # [128, 32]

    shared = {"wq": wq, "wkv": wkv, "wp": wp, "idt": idt, "idtf": idtf}
    if has_qb:
        qbr = q_b.reshape(NH, KD).T.reshape(NH * KD)
        shared["qb"] = np.broadcast_to(qbr.astype(bf), (P, NH * KD)).copy()
    if has_kvb:
        shared["kvb"] = np.broadcast_to(kv_b.astype(bf), (P, KD + VD)).copy()
    if has_pb:
        shared["pb"] = (proj_b * layer_scale.reshape(-1)).reshape(C, 1) \
            .astype(np.float32)
    return shared


def kernel(x, q_w, q_b, kv_w, kv_b, proj_w, proj_b, layer_scale):
    from concourse.bass_utils import run_bass_kernel_spmd
    import os

    x = np.asarray(x, dtype=np.float32)
    q_w = np.asarray(q_w, dtype=np.float32)
    q_b = np.asarray(q_b, dtype=np.float32)
    kv_w = np.asarray(kv_w, dtype=np.float32)
    kv_b = np.asarray(kv_b, dtype=np.float32)
    proj_w = np.asarray(proj_w, dtype=np.float32)
    proj_b = np.asarray(proj_b, dtype=np.float32)
    layer_scale = np.asarray(layer_scale, dtype=np.float32)

    has_qb = bool(np.any(q_b != 0))
    has_kvb = bool(np.any(kv_b != 0))
    has_pb = bool(np.any(proj_b != 0))
    nc = _get_nc(has_qb, has_kvb, has_pb)

    bf = ml_dtypes.bfloat16
    shared = _host_inputs(q_w, q_b, kv_w, kv_b, proj_w, proj_b, layer_scale,
                          has_qb, has_kvb, has_pb)

    in_maps = []
    for c in range(NCORES):
        b, hg = c // 4, c % 4
        xc = np.ascontiguousarray(
            x[b, :, :, hg * HS:(hg + 1) * HS, :].reshape(C, T)).astype(bf)
        in_maps.append({"x": xc, **shared})

    trace = bool(int(os.environ.get("KERNEL_TRACE", "0")))
    res = run_bass_kernel_spmd(nc, in_maps, core_ids=list(range(NCORES)),
                               trace=trace)
    kernel.last_results = res

    out = np.empty((B, C, D, H, W), dtype=np.float32)
    for c in range(NCORES):
        b, hg = c // 4, c % 4
        # out token order: t = mu*4096 + w'*128 + dqloc*8 + n
        r = res.results[c]["out"].reshape(C, 2, W, 16, NH)
        for mu in range(2):
            out[b, :, mu * 16:(mu + 1) * 16, hg::4, :] = \
                r[:, mu].transpose(0, 2, 3, 1)
    return out
